# revision 36
# baseline (speedup 1.0000x reference)
"""Causal grouped self-attention (GQA) for Trainium2, 8 NeuronCores.

Sharding: core c = 4*b + g handles batch b and kv-head group g
(q heads 4g..4g+3, kv head g). Each core computes its 4 heads' attention
output and a partial out-projection over its 512 input channels; the host
sums the 4 partials per batch.

Per-core layout strategy:
  - x fed pre-transposed as xT (C, T); projections produce qT/kT in
    (head_dim, t) layout directly (stationary = weight chunks).
  - scores computed transposed: sT[k, t] = sum_d kT[d,k] qT[d,t] so the
    softmax column-sum is a ones-matmul on the PE and attn@v directly
    yields yT (d, t) with v in natural (k, d) layout.
  - RMS-norm scales: rstd_k applied via the per-partition `scale` operand
    of the exp activation; rstd_q (and 1/sqrt(dh)) applied to qT via a
    ones-row outer-product broadcast matmul.
  - causal mask added into the score psum via identity-matmul of a
    precomputed -1e30 step tile; exp(-huge) == 0 exactly.
"""
import os
from contextlib import ExitStack

import numpy as np

import concourse.bass as bass
import concourse.mybir as mybir
import concourse.tile as tile
from concourse import bacc
from concourse.bass_utils import run_bass_kernel_spmd
from concourse.masks import make_identity

# Force every activation onto the one ACT table set that contains Exp, Ln,
# Square and Copy together ("natural_log_exp_and_others"); the default
# chooser alternates exp_and_others <-> natural_log and pays a ~2.7us
# table reload per switch, dozens of times per kernel.
if not getattr(bacc, "_attn_act_tables_patched", False):
    _orig_gat = bacc.get_activation_tables

    def _gat_single_set(arch):
        tables = _orig_gat(arch)
        keep = "natural_log_exp_and_others"
        return {n: (f if n == keep else set()) for n, f in tables.items()}

    bacc.get_activation_tables = _gat_single_set
    bacc._attn_act_tables_patched = True

F32 = mybir.dt.float32
F32R = mybir.dt.float32r
AF = mybir.ActivationFunctionType

B = 2
T = 2048
C = 2048
DH = 128
NH = 16
NKV = 4
HQ = NH // NKV          # 4 local q heads per core
NTB = T // 512          # 4 t-blocks
TBS = 512
NKC = C // 128          # 16 contraction chunks
RMS_EPS = 1.1920929e-07
MASK_VAL = -1.0e30

# matmul operand dtype:
#   "f32r" (default): hardware-rounded fp32 (TF32-like, ~13-bit mantissa),
#          end-to-end error ~2.4e-4 absmax/scale, ~272us/core
#   "f32": exact fp32 matmuls, error ~2.8e-6, ~1000us/core
MM_DTYPE = os.environ.get("ATTN_MM_DTYPE", "f32r")


def build_program(dt_name: str):
    DT = {"f32": F32, "f32r": F32R}[dt_name]
    nc = bacc.Bacc("TRN2", target_bir_lowering=False, debug=False)

    xT = nc.dram_tensor("xT", [C, T], DT, kind="ExternalInput")
    wq = nc.dram_tensor("wq", [C, HQ * DH], DT, kind="ExternalInput")
    wk = nc.dram_tensor("wk", [C, DH], DT, kind="ExternalInput")
    wv = nc.dram_tensor("wv", [C, DH], DT, kind="ExternalInput")
    wp = nc.dram_tensor("wp", [HQ * DH, C], DT, kind="ExternalInput")
    cqt = nc.dram_tensor("cqt", [128, HQ], F32, kind="ExternalInput")
    sqt = nc.dram_tensor("sqt", [128, HQ], F32, kind="ExternalInput")
    ckt = nc.dram_tensor("ckt", [128, 1], F32, kind="ExternalInput")
    skt = nc.dram_tensor("skt", [128, 1], F32, kind="ExternalInput")
    maskt = nc.dram_tensor("maskt", [128, TBS], DT, kind="ExternalInput")
    maskt3 = nc.dram_tensor("maskt3", [128, 256], DT, kind="ExternalInput")
    identt = nc.dram_tensor("identt", [128, 128], DT, kind="ExternalInput")
    onest = nc.dram_tensor("onest", [128, 1], DT, kind="ExternalInput")
    onesrt = nc.dram_tensor("onesrt", [1, 128], F32, kind="ExternalInput")
    onesrdt = nc.dram_tensor("onesrdt", [1, 128], DT, kind="ExternalInput")
    out = nc.dram_tensor("out", [T, C], F32, kind="ExternalOutput")

    def rope(psq, dst, cos2, sin2pm, bc, sca):
        """dst = rope(psq) [* bc].  psq: (128,TBS) psum (d on partitions);
        cos2: (128,1) sbuf cos table duplicated across halves; sin2pm:
        (128,1) sbuf sin table, rows 64:128 NEGATED (sign folded in);
        bc: (128,TBS) psum broadcast scale or None."""
        tA = sca.tile([128, TBS], F32, tag="tA")
        tB = sca.tile([128, TBS], F32, tag="tB")
        nc.vector.tensor_scalar_mul(tA[:], psq[:], cos2[:])
        nc.vector.tensor_scalar_mul(tB[0:64, :], psq[64:128, :], sin2pm[0:64, :])
        nc.vector.tensor_scalar_mul(tB[64:128, :], psq[0:64, :], sin2pm[64:128, :])
        if bc is None:
            nc.vector.tensor_add(dst[:], tA[:], tB[:])
        else:
            rot = sca.tile([128, TBS], F32, tag="rot")
            nc.vector.tensor_add(rot[:], tA[:], tB[:])
            nc.vector.tensor_mul(dst[:], rot[:], bc[:])

    with tile.TileContext(nc) as tc, ExitStack() as stk:
        wpool = stk.enter_context(tc.tile_pool(name="w", bufs=1))
        data = stk.enter_context(tc.tile_pool(name="data", bufs=1))
        qtp = stk.enter_context(tc.tile_pool(name="qtp", bufs=5))
        xtp = stk.enter_context(tc.tile_pool(name="xtp", bufs=16))
        expp = stk.enter_context(tc.tile_pool(name="expp", bufs=5))
        ytp = stk.enter_context(tc.tile_pool(name="ytp", bufs=4))
        sca = stk.enter_context(tc.tile_pool(name="sca", bufs=2))
        rows = stk.enter_context(tc.tile_pool(name="rows", bufs=3))
        rowd = stk.enter_context(tc.tile_pool(name="rowd", bufs=2))
        osp = stk.enter_context(tc.tile_pool(name="osp", bufs=4))
        psMM = stk.enter_context(tc.tile_pool(name="psMM", bufs=4, space="PSUM"))
        psDen = stk.enter_context(tc.tile_pool(name="psDen", bufs=2, space="PSUM"))
        psY = stk.enter_context(tc.tile_pool(name="psY", bufs=2, space="PSUM"))

        # ---- static loads -------------------------------------------------
        # k/v weights first (small, unblock phase A), wq per-chunk so the
        # first projection matmuls start as soon as their chunk lands,
        # wp deferred to just before the first out-projection.
        wk_s = wpool.tile([128, NKC, DH], DT)
        wv_s = wpool.tile([128, NKC, DH], DT)
        wq_cs = [wpool.tile([128, HQ * DH], DT, tag=f"wq{kc}", name=f"wq_c{kc}")
                 for kc in range(NKC)]
        wp_s = wpool.tile([128, HQ, C], DT)
        mask_s = wpool.tile([128, TBS], DT)
        mask3_s = wpool.tile([128, 256], DT)
        cq_s = wpool.tile([128, HQ], F32)
        nc.sync.dma_start(out=cq_s[:], in_=cqt[:])
        sq_s = wpool.tile([128, HQ], F32)
        nc.sync.dma_start(out=sq_s[:], in_=sqt[:])
        ck_s = wpool.tile([128, 1], F32)
        nc.sync.dma_start(out=ck_s[:], in_=ckt[:])
        sk_s = wpool.tile([128, 1], F32)
        nc.sync.dma_start(out=sk_s[:], in_=skt[:])
        ident_s = wpool.tile([128, 128], DT)
        onesc = wpool.tile([128, 1], DT)
        nc.sync.dma_start(out=onesc[:], in_=onest[:])
        ones_row = wpool.tile([1, 128], F32)
        nc.sync.dma_start(out=ones_row[:], in_=onesrt[:])
        ones_row_dt = wpool.tile([1, 128], DT)
        nc.sync.dma_start(out=ones_row_dt[:], in_=onesrdt[:])
        ident128f = wpool.tile([128, 128], F32)
        make_identity(nc, ident128f[:])
        ident16 = wpool.tile([16, 16], F32)
        make_identity(nc, ident16[:])
        epsb = wpool.tile([1, 1], F32)
        nc.vector.memset(epsb[:], RMS_EPS)
        b0q = wpool.tile([1, 1], F32)
        nc.vector.memset(b0q[:], float(-0.5 * np.log(DH)))

        rk16 = data.tile([16, 128], F32, tag="rk16")
        nc.vector.memset(rk16[:], 1.0)   # regrouped for transpose
        rkc_s = data.tile([128, 16], F32, tag="rkc")   # rstd_k as columns

        kts = []   # (128, TBS) per t-block:  kT (roped, unnormalized)
        vts = []   # (128, 128) per k-tile:   v natural (k, d)

        for j in range(NTB):
            # ---- phase A: projections -------------------------------------
            xts = []
            for kc in range(NKC):
                t_ = xtp.tile([128, TBS], DT, tag="xt")
                nc.sync.dma_start(
                    out=t_[:],
                    in_=xT[128 * kc : 128 * (kc + 1), TBS * j : TBS * (j + 1)],
                )
                xts.append(t_)
                if j == 0:
                    eng = nc.sync if kc % 2 == 0 else nc.scalar
                    eng.dma_start(out=wq_cs[kc][:],
                                  in_=wq[128 * kc : 128 * (kc + 1), :])

            if j == 0:
                # big weight loads after the first x tiles so the PE can
                # start on q-projection almost immediately
                nc.sync.dma_start(out=wv_s[:],
                                  in_=wv[:].rearrange("(kc p) m -> p kc m", p=128))
                nc.sync.dma_start(out=wk_s[:],
                                  in_=wk[:].rearrange("(kc p) m -> p kc m", p=128))
                nc.sync.dma_start(out=mask_s[:], in_=maskt[:])
                nc.sync.dma_start(out=mask3_s[:], in_=maskt3[:])
                nc.sync.dma_start(out=ident_s[:], in_=identt[:])

            # q/v/k projection units, two-stage software pipeline:
            #   proj(u) | post1(u-1): ACT chains + sumsq-MM + psum evict
            #           | post2(u-2): broadcast-MM + rope (inputs long ready)
            qts = [None] * HQ
            kt = data.tile([128, TBS], DT, tag=f"kt{j}", name=f"kt{j}")

            def proj_q(h):
                psq = psMM.tile([128, TBS], F32, tag="pmm", name="psq")
                for kc in range(NKC):
                    nc.tensor.matmul(psq[:], wq_cs[kc][:, DH * h : DH * (h + 1)],
                                     xts[kc][:],
                                     start=(kc == 0), stop=(kc == NKC - 1))
                return psq

            def post1_q(h, psq):
                sqq = sca.tile([128, TBS], DT, tag="sq", name="sqq")
                nc.scalar.activation(sqq[:], psq[:], AF.Square)
                qraw = sca.tile([128, TBS], F32, tag="qraw", name="qraw")
                nc.scalar.copy(qraw[:], psq[:])
                ssq = psDen.tile([1, TBS], F32, tag="den", name="ssq")
                nc.tensor.matmul(ssq[:], onesc[:], sqq[:], start=True, stop=True)
                lnq = rows.tile([1, TBS], F32, tag="rowtmp", name="lnq")
                nc.scalar.activation(lnq[:], ssq[:], AF.Ln,
                                     bias=epsb[:], scale=1.0 / DH)
                rqr = rowd.tile([1, TBS], DT, tag="rowtmpd", name="rqr")
                nc.scalar.activation(rqr[:], lnq[:], AF.Exp,
                                     bias=b0q[:], scale=-0.5)
                return (qraw, rqr)

            def post2_q(h, ctx):
                qraw, rqr = ctx
                bcq = psDen.tile([128, TBS], F32, tag="den", name="bcq")
                nc.tensor.matmul(bcq[:], ones_row_dt[:], rqr[:],
                                 start=True, stop=True)
                qt = qtp.tile([128, TBS], DT, tag="qt", name="qt")
                rope(qraw, qt, cq_s[:, h : h + 1], sq_s[:, h : h + 1], bcq, sca)
                qts[h] = qt

            def proj_v(_):
                psvT = psMM.tile([128, TBS], F32, tag="pmm", name="psvT")
                for kc in range(NKC):
                    nc.tensor.matmul(psvT[:], wv_s[:, kc, :], xts[kc][:],
                                     start=(kc == 0), stop=(kc == NKC - 1))
                return psvT

            def post1_v(_, psvT):
                vT_s = sca.tile([128, TBS], F32, tag="vT", name="vT_s")
                nc.scalar.copy(vT_s[:], psvT[:])
                return vT_s

            def post2_v(_, vT_s):
                for tt in range(4):
                    i = 4 * j + tt
                    ptv = psY.tile([128, 128], F32, tag="y", name="ptv")
                    nc.tensor.transpose(ptv[:], vT_s[:, 128 * tt : 128 * (tt + 1)],
                                        ident128f[:])
                    vt = data.tile([128, 128], DT, tag=f"v{i}", name=f"v{i}")
                    nc.vector.tensor_copy(vt[:], ptv[:])
                    vts.append(vt)

            def proj_k(_):
                psk = psMM.tile([128, TBS], F32, tag="pmm", name="psk")
                for kc in range(NKC):
                    nc.tensor.matmul(psk[:], wk_s[:, kc, :], xts[kc][:],
                                     start=(kc == 0), stop=(kc == NKC - 1))
                return psk

            def post1_k(_, psk):
                sqk = sca.tile([128, TBS], DT, tag="sq", name="sqk")
                nc.scalar.activation(sqk[:], psk[:], AF.Square)
                kraw = sca.tile([128, TBS], F32, tag="qraw", name="kraw")
                nc.scalar.copy(kraw[:], psk[:])
                ssk = psDen.tile([1, TBS], F32, tag="den", name="ssk")
                nc.tensor.matmul(ssk[:], onesc[:], sqk[:], start=True, stop=True)
                lnk = rows.tile([1, TBS], F32, tag="rowtmp", name="lnk")
                nc.scalar.activation(lnk[:], ssk[:], AF.Ln,
                                     bias=epsb[:], scale=1.0 / DH)
                rk_row = rows.tile([1, TBS], F32, tag="rowtmp", name="rk_row")
                nc.scalar.activation(rk_row[:], lnk[:], AF.Exp,
                                     bias=0.0, scale=-0.5)
                return (kraw, rk_row)

            def post2_k(_, ctx):
                kraw, rk_row = ctx
                rope(kraw, kt, ck_s, sk_s, None, sca)
                nc.gpsimd.dma_start(
                    out=rk16[4 * j : 4 * (j + 1), :],
                    in_=rk_row[0:1, :].rearrange("a (c d) -> a c d", d=128),
                )
                ptr = psY.tile([128, 16], F32, tag="y", name="ptr")
                nc.tensor.transpose(ptr[:], rk16[:], ident16[:])
                nc.scalar.copy(rkc_s[:], ptr[:])

            if j == 0:
                # wk lands after the interleaved xt/wq stream: keep k mid
                units = [(proj_q, post1_q, post2_q, 0),
                         (proj_q, post1_q, post2_q, 1),
                         (proj_k, post1_k, post2_k, None),
                         (proj_q, post1_q, post2_q, 2),
                         (proj_q, post1_q, post2_q, 3),
                         (proj_v, post1_v, post2_v, None)]
            else:
                # k first: its rstd/rope chain gates phase B's first exps
                units = [(proj_k, post1_k, post2_k, None),
                         (proj_q, post1_q, post2_q, 0),
                         (proj_q, post1_q, post2_q, 1),
                         (proj_q, post1_q, post2_q, 2),
                         (proj_q, post1_q, post2_q, 3),
                         (proj_v, post1_v, post2_v, None)]
            s1 = None   # (post1, arg, psum)
            s2 = None   # (post2, arg, ctx)
            for pr, po1, po2, arg in units:
                ps = pr(arg)
                if s2 is not None:
                    s2[0](s2[1], s2[2])
                if s1 is not None:
                    s2 = (s1[3], s1[1], s1[0](s1[1], s1[2]))
                else:
                    s2 = None
                s1 = (po1, arg, ps, po2)
            if s2 is not None:
                s2[0](s2[1], s2[2])
            s2 = (s1[3], s1[1], s1[0](s1[1], s1[2]))
            s2[0](s2[1], s2[2])
            kts.append(kt)

            # ---- phase B: attention per head ------------------------------
            def finish_norm(den, psy):
                dr = rows.tile([1, TBS], F32, tag="rowtmp", name="dr")
                if DT == F32R:
                    nc.vector.reciprocal_approx_fast(out=dr[:], in_=den[:])
                else:
                    dscr = rows.tile([1, TBS], F32, tag="rowtmp", name="dscr")
                    nc.vector.reciprocal_approx_accurate(out=dr[:], in_=den[:],
                                                         scratch=dscr[:])
                if DT != F32:
                    drd = rowd.tile([1, TBS], DT, tag="rowtmpd", name="drd")
                    nc.vector.tensor_copy(drd[:], dr[:])
                else:
                    drd = dr
                bcd = psDen.tile([128, TBS], F32, tag="den", name="bcd")
                nc.tensor.matmul(bcd[:], ones_row_dt[:], drd[:], start=True, stop=True)
                bcs = sca.tile([128, TBS], F32, tag="bcs", name="bcs")
                nc.vector.tensor_copy(bcs[:], bcd[:])
                yt = ytp.tile([128, TBS], DT, tag="yt", name="yt")
                nc.vector.tensor_mul(yt[:], psy[:], bcs[:])
                return yt

            yts = []
            pending = []
            nkt = 4 * (j + 1)
            for h in range(HQ):
                den = psDen.tile([1, TBS], F32, tag="den")
                psy = psY.tile([128, TBS], F32, tag="y")
                for i in range(nkt):
                    diag = i >= 4 * j
                    m = i - 4 * j
                    # first causal t column; m=3 padded to N=256 so the
                    # f32r matmuls stay on the 1 cyc/row path (N>=256)
                    lo = (256 if m == 3 else 128 * m) if diag else 0
                    pss = psMM.tile([128, TBS], F32, tag="pmm")
                    nc.tensor.matmul(pss[:, lo:],
                                     kts[i // 4][:, 128 * (i % 4) : 128 * (i % 4 + 1)],
                                     qts[h][:, lo:], start=True, stop=True)
                    if diag:
                        mask_ap = mask3_s[:, :] if m == 3 else mask_s[:, : TBS - lo]
                        nc.vector.tensor_add(pss[:, lo:], pss[:, lo:], mask_ap)
                    e = expp.tile([128, TBS], DT, tag="e")
                    nc.scalar.activation(e[:, lo:], pss[:, lo:], AF.Exp,
                                         bias=0.0, scale=rkc_s[:, i : i + 1])
                    nc.tensor.matmul(den[:, lo:], onesc[:], e[:, lo:],
                                     start=(i == 0), stop=(i == nkt - 1),
                                     skip_group_check=True)
                    nc.tensor.matmul(psy[:, lo:], vts[i][:], e[:, lo:],
                                     start=(i == 0), stop=(i == nkt - 1),
                                     skip_group_check=True)
                pending.append((den, psy))
                if len(pending) > 1:
                    yts.append(finish_norm(*pending.pop(0)))

            while pending:
                yts.append(finish_norm(*pending.pop(0)))

            # ---- phase C: partial out-projection --------------------------
            if j == 0:
                nc.sync.dma_start(
                    out=wp_s[:], in_=wp[:].rearrange("(kc p) m -> p kc m", p=128))
            for tt in range(4):
                for nb in range(4):
                    po = psMM.tile([128, TBS], F32, tag="pmm")
                    for h in range(HQ):
                        nc.tensor.matmul(po[:],
                                         yts[h][:, 128 * tt : 128 * (tt + 1)],
                                         wp_s[:, h, TBS * nb : TBS * (nb + 1)],
                                         start=(h == 0), stop=(h == HQ - 1))
                    os_ = osp.tile([128, TBS], F32, tag="os")
                    nc.vector.tensor_copy(os_[:], po[:])
                    nc.sync.dma_start(
                        out=out[TBS * j + 128 * tt : TBS * j + 128 * (tt + 1),
                                TBS * nb : TBS * (nb + 1)],
                        in_=os_[:],
                    )

    nc.compile()
    return nc


def make_inputs_for_core(c, x, Wq, Wkv, Wproj):
    """Host-side shard prep for core c = 4*b + g."""
    b, g = c // 4, c % 4
    f32 = np.float32
    xT_b = np.ascontiguousarray(x[b].T.astype(f32))
    wq_c = np.ascontiguousarray(Wq[512 * g : 512 * (g + 1), :].T.astype(f32))
    wk_c = np.ascontiguousarray(Wkv[DH * g : DH * (g + 1), :].T.astype(f32))
    wv_c = np.ascontiguousarray(
        Wkv[NKV * DH + DH * g : NKV * DH + DH * (g + 1), :].T.astype(f32))
    wp_c = np.ascontiguousarray(Wproj[:, 512 * g : 512 * (g + 1)].T.astype(f32))

    af = (1.0 / 1024.0) ** np.linspace(0.0, 1.0, DH // 4, dtype=f32)
    freqs = np.concatenate([af, np.zeros(DH // 4, f32)])  # (64,)
    cqt = np.zeros((128, HQ), f32)
    sqt = np.zeros((128, HQ), f32)
    for h in range(HQ):
        th = (4 * g + h) * freqs
        cqt[0:64, h] = np.cos(th)
        cqt[64:128, h] = np.cos(th)
        sqt[0:64, h] = np.sin(th)
        sqt[64:128, h] = -np.sin(th)
    thk = g * freqs
    ckt = np.concatenate([np.cos(thk), np.cos(thk)]).reshape(128, 1).astype(f32)
    skt = np.concatenate([np.sin(thk), -np.sin(thk)]).reshape(128, 1).astype(f32)

    a = np.arange(128)[:, None]
    t = np.arange(TBS)[None, :]
    maskt = np.where(a > t, np.float32(MASK_VAL), np.float32(0.0))
    u = np.arange(256)[None, :]
    maskt3 = np.where(a + 128 > u, np.float32(MASK_VAL), np.float32(0.0))

    return {
        "xT": xT_b,
        "wq": wq_c,
        "wk": wk_c,
        "wv": wv_c,
        "wp": wp_c,
        "cqt": cqt,
        "sqt": sqt,
        "ckt": ckt,
        "skt": skt,
        "maskt": maskt.astype(f32),
        "maskt3": maskt3.astype(f32),
        "identt": np.eye(128, dtype=f32),
        "onest": np.ones((128, 1), f32),
        "onesrt": np.ones((1, 128), f32),
        "onesrdt": np.ones((1, 128), f32),
    }


_CACHE = {}


def _get_program():
    key = MM_DTYPE
    if key not in _CACHE:
        _CACHE[key] = build_program(key)
    return _CACHE[key]


def kernel(x, Wq, Wkv, Wproj, tok_masks=None, **_ignored):
    x = np.asarray(x, np.float32)
    Wq = np.asarray(Wq, np.float32)
    Wkv = np.asarray(Wkv, np.float32)
    Wproj = np.asarray(Wproj, np.float32)

    nc = _get_program()
    in_maps = [make_inputs_for_core(c, x, Wq, Wkv, Wproj) for c in range(8)]
    res = run_bass_kernel_spmd(nc, in_maps, list(range(8)))
    parts = [res.results[c]["out"] for c in range(8)]
    y = np.stack([
        parts[0] + parts[1] + parts[2] + parts[3],
        parts[4] + parts[5] + parts[6] + parts[7],
    ]).astype(np.float32)
    return y, np.float32(0.0)
